# revision 11
# baseline (speedup 1.0000x reference)
"""Multi-head attention kernel for Trainium2, SPMD across 8 NeuronCores.

Problem: x[8,16,256,384] -> attention(8 heads, head_dim 64) -> [8,16,256,384]
Sharding: data-parallel over batch b (1 batch element per core, weights
replicated). Each core processes 16 independent slices of [256 tokens, 384],
handled in pairs ("superslices") so the QKV matmuls stream N=512.

Per-slice dataflow (activations kept feature-major, i.e. transposed):
  xT[d,t]   = DMA-transpose load of x slice         [384, 256] (bf16)
  qkT[e,t]  = w_qkv[:,e].T @ xT  (e in 0..1023)     q^T,k^T feature-major
  v[t,e]    = xT.T @ w_qkv[:, 1024:1536]            natural layout
  sT[j,i]   = k_h^T.T @ q_h^T   (per head, K=64; head pairs row-tiled)
  pT        = exp(sT / 8)                           [j, 2 heads x i]
  rowsum    = ones.T @ pT                           [1, 512] matmul
  o         = v_h.T @ pT  (head pair col-tiled into one [128,256] PSUM)
  oT        = o * broadcast(1/rowsum)               one mul per head pair
  out[t,:]  = oT.T @ w_out + b_out (bias via K=1 ones matmul)
"""

import sys
import types

sys.path.insert(0, "/opt/trn_rl_repo")

import numpy as np

import concourse.bass as bass
import concourse.bacc as bacc
import concourse.mybir as mybir
import concourse.tile as tile
from concourse.bass_utils import run_bass_kernel_spmd

N_CORES = 8
B, P, N, D = 8, 16, 256, 384
H, HD = 8, 64
INNER = H * HD  # 512
SCALE = HD ** -0.5
F32 = mybir.dt.float32

MM_MODE = "bf16"  # "bf16" | "f32r" | "f32"


def _mdt(mm_mode):
    return {"bf16": mybir.dt.bfloat16,
            "f32r": mybir.dt.float32r,
            "f32": F32}[mm_mode]


def _np_mdt(mm_mode):
    if mm_mode == "bf16":
        import ml_dtypes
        return ml_dtypes.bfloat16
    return np.float32


def _register_ntff_hook():
    """Make trace=True work under axon when antenv.axon_hooks is absent."""
    if "antenv.axon_hooks" in sys.modules:
        return
    try:
        from trn_agent_boot.trn_boot import _ntff_profile_via_ctypes
    except ImportError:
        return
    hook = _ntff_profile_via_ctypes("/opt/axon/libaxon_pjrt.so")
    mod = types.ModuleType("antenv.axon_hooks")
    mod.get_axon_ntff_profile_hook = lambda: hook
    sys.modules["antenv.axon_hooks"] = mod


def build(mm_mode=MM_MODE):
    nc = bacc.Bacc("TRN2", target_bir_lowering=False, debug=False,
                   num_devices=N_CORES)
    MDT = _mdt(mm_mode)
    x_ext = nc.declare_dram_parameter("x", [P, N, D], MDT, isOutput=False)
    wq_ext = nc.declare_dram_parameter("w_qkv", [D, 3 * INNER], MDT,
                                       isOutput=False)
    wo_ext = nc.declare_dram_parameter("w_out", [INNER, D], MDT,
                                       isOutput=False)
    bo_ext = nc.declare_dram_parameter("b_out", [D], MDT, isOutput=False)
    out_ext = nc.declare_dram_parameter("out", [P, N, D], F32, isOutput=True)

    Exp = mybir.ActivationFunctionType.Exp
    memset_dt = F32 if mm_mode != "bf16" else MDT

    with tile.TileContext(nc) as tc:
        with (
            tc.tile_pool(name="const", bufs=1) as const,
            tc.tile_pool(name="xt", bufs=2) as xt_pool,
            tc.tile_pool(name="qk", bufs=2) as qk_pool,
            tc.tile_pool(name="vp", bufs=2) as v_pool,
            tc.tile_pool(name="pt", bufs=6) as p_pool,
            tc.tile_pool(name="ot", bufs=3) as ot_pool,
            tc.tile_pool(name="ob", bufs=3) as ob_pool,
            tc.tile_pool(name="rs", bufs=6) as rs_pool,
            tc.tile_pool(name="bc", bufs=6) as bc_pool,
            tc.tile_pool(name="mmps", bufs=2, space="PSUM") as mm_ps,
            tc.tile_pool(name="sps", bufs=2, space="PSUM") as s_ps,
            tc.tile_pool(name="ops", bufs=2, space="PSUM") as o_ps,
            tc.tile_pool(name="rps", bufs=1, space="PSUM") as r_ps,
            tc.tile_pool(name="bps", bufs=1, space="PSUM") as b_ps,
        ):
            # ---- constants (loaded once) ----
            w_sb = const.tile([128, 3 * 1536], MDT, tag="w_sb")
            for kc in range(3):
                nc.sync.dma_start(w_sb[:, kc * 1536:(kc + 1) * 1536],
                                  wq_ext.ap()[kc * 128:(kc + 1) * 128, :])
            wo_sb = const.tile([128, 4 * 384], MDT, tag="wo_sb")
            for kc in range(4):
                nc.sync.dma_start(wo_sb[:, kc * 384:(kc + 1) * 384],
                                  wo_ext.ap()[kc * 128:(kc + 1) * 128, :])
            bt_sb = const.tile([1, 384], MDT, tag="bt_sb")
            nc.sync.dma_start(bt_sb[:], bo_ext.ap().unsqueeze(0))
            on_sb = const.tile([128, 128], MDT, tag="on_sb")
            nc.gpsimd.memset(on_sb[:], 1.0)
            ind0 = const.tile([1, 128], MDT, tag="ind0")
            nc.gpsimd.memset(ind0[:, 0:64], 1.0)
            nc.gpsimd.memset(ind0[:, 64:128], 0.0)
            ind1 = const.tile([1, 128], MDT, tag="ind1")
            nc.gpsimd.memset(ind1[:, 0:64], 0.0)
            nc.gpsimd.memset(ind1[:, 64:128], 1.0)

            # m-chunk order: interleave q and k chunks so head-pair c has
            # its q (m=c) and k (m=4+c) chunks available early.
            m_order = [0, 4, 1, 5, 2, 6, 3, 7]

            for u in range(P // 2):  # superslice of 2 token slices
                # ---- xT via DMA transpose: [256,128] dram -> [128,256] ----
                xt = xt_pool.tile([128, 3 * 512], MDT, tag="xt")
                for a in range(2):
                    for kc in range(3):
                        nc.scalar.dma_start(
                            xt[:, kc * 512 + a * 256: kc * 512 + (a + 1) * 256],
                            x_ext.ap()[2 * u + a, :, kc * 128:(kc + 1) * 128],
                            transpose=True)

                # ---- qkT chunks m (features m*128..m*128+127) ----
                qk = qk_pool.tile([128, 8 * 512], MDT, tag="qk")
                for mi, m in enumerate(m_order):
                    ps = mm_ps.tile([128, 512], F32, tag="mmps")
                    for kc in range(3):
                        nc.tensor.matmul(
                            ps[:],
                            w_sb[:, kc * 1536 + m * 128:
                                 kc * 1536 + (m + 1) * 128],
                            xt[:, kc * 512:(kc + 1) * 512],
                            start=(kc == 0), stop=(kc == 2))
                    if mi % 2 == 0:
                        nc.scalar.copy(qk[:, m * 512:(m + 1) * 512], ps[:])
                    else:
                        nc.vector.tensor_copy(qk[:, m * 512:(m + 1) * 512],
                                              ps[:])

                # ---- v natural [token, 512] per (slice, tok-chunk) ----
                v = v_pool.tile([128, 4 * 512], MDT, tag="v")
                for a in range(2):
                    for t in range(2):
                        ps = mm_ps.tile([128, 512], F32, tag="mmps")
                        for kc in range(3):
                            nc.tensor.matmul(
                                ps[:],
                                xt[:, kc * 512 + a * 256 + t * 128:
                                   kc * 512 + a * 256 + (t + 1) * 128],
                                w_sb[:, kc * 1536 + 1024: kc * 1536 + 1536],
                                start=(kc == 0), stop=(kc == 2))
                        nc.vector.tensor_copy(
                            v[:, (a * 2 + t) * 512:(a * 2 + t + 1) * 512],
                            ps[:])

                # ---- attention: head pairs (2c, 2c+1) per slice ----
                for a in range(2):
                    ot = ot_pool.tile([128, 4 * 256], MDT, tag="ot")
                    for c in range(4):
                        pts = []
                        for jc in range(2):
                            pt = p_pool.tile([128, 512], MDT, tag="pt")
                            for e in range(2):
                                sps = s_ps.tile([128, 256], F32, tag="sps")
                                nc.tensor.matmul(
                                    sps[:],
                                    qk[e * 64:e * 64 + 64,
                                       (4 + c) * 512 + a * 256 + jc * 128:
                                       (4 + c) * 512 + a * 256 + (jc + 1) * 128],
                                    qk[e * 64:e * 64 + 64,
                                       c * 512 + a * 256: c * 512 + (a + 1) * 256],
                                    start=True, stop=True,
                                    tile_position=(e * 64, 0))
                                nc.scalar.activation(
                                    pt[:, e * 256:(e + 1) * 256], sps[:], Exp,
                                    scale=SCALE)
                            pts.append(pt)
                        # rowsums for both heads: [1, 512] = ones.T @ pT
                        rps = r_ps.tile([1, 512], F32, tag="rps")
                        for jc in range(2):
                            nc.tensor.matmul(
                                rps[:], on_sb[:, 0:1], pts[jc][:],
                                start=(jc == 0), stop=(jc == 1))
                        # AV pair, col-tiled into one PSUM tile
                        ops = o_ps.tile([128, 256], F32, tag="ops")
                        for jc in range(2):
                            for e in range(2):
                                h = 2 * c + e
                                nc.tensor.matmul(
                                    ops[e * 64:(e + 1) * 64, :],
                                    v[:, (a * 2 + jc) * 512 + h * 64:
                                      (a * 2 + jc) * 512 + (h + 1) * 64],
                                    pts[jc][:, e * 256:(e + 1) * 256],
                                    start=(jc == 0), stop=(jc == 1),
                                    tile_position=(0, e * 64))
                        rs = rs_pool.tile([1, 512], MDT, tag="rs")
                        with nc.allow_low_precision(
                                reason="softmax denom recip in bf16"):
                            nc.vector.reciprocal(rs[:], rps[:])
                        bps = b_ps.tile([128, 256], F32, tag="bps")
                        nc.tensor.matmul(bps[:], ind0[:], rs[0:1, 0:256],
                                         start=True, stop=False)
                        nc.tensor.matmul(bps[:], ind1[:], rs[0:1, 256:512],
                                         start=False, stop=True)
                        bc = bc_pool.tile([128, 256], F32, tag="bc")
                        nc.scalar.copy(bc[:], bps[:])
                        nc.vector.tensor_mul(ot[:, c * 256:(c + 1) * 256],
                                             ops[:], bc[:])

                    # ---- output projection; bias via K=1 ones matmul ----
                    for t in range(2):
                        fps = mm_ps.tile([128, 512], F32, tag="mmps")
                        for kc in range(4):
                            nc.tensor.matmul(
                                fps[:, 0:384],
                                ot[:, kc * 256 + t * 128:
                                   kc * 256 + (t + 1) * 128],
                                wo_sb[:, kc * 384:(kc + 1) * 384],
                                start=(kc == 0), stop=False)
                        nc.tensor.matmul(
                            fps[:, 0:384], on_sb[0:1, 0:128], bt_sb[:],
                            start=False, stop=True)
                        ob = ob_pool.tile([128, 384], F32, tag="ob")
                        nc.scalar.copy(ob[:], fps[:, 0:384])
                        nc.sync.dma_start(
                            out_ext.ap()[2 * u + a, t * 128:(t + 1) * 128, :],
                            ob[:])
    nc.compile()
    return nc


_CACHE = {}


def _get_nc(mm_mode=MM_MODE):
    if mm_mode not in _CACHE:
        _CACHE[mm_mode] = build(mm_mode)
    return _CACHE[mm_mode]


def _in_maps(inputs, mm_mode=MM_MODE):
    ndt = _np_mdt(mm_mode)
    x = np.asarray(inputs["x"]).astype(ndt)
    w_qkv = np.asarray(inputs["w_qkv"]).astype(ndt)
    w_out = np.asarray(inputs["w_out"]).astype(ndt)
    b_out = np.asarray(inputs["b_out"]).astype(ndt)
    return [
        {"x": np.ascontiguousarray(x[i]), "w_qkv": w_qkv, "w_out": w_out,
         "b_out": b_out}
        for i in range(N_CORES)
    ]


def run(inputs, trace=False, mm_mode=MM_MODE):
    """Returns (output [8,16,256,384], exec_time_ns or None)."""
    if trace:
        _register_ntff_hook()
    nc = _get_nc(mm_mode)
    res = run_bass_kernel_spmd(nc, _in_maps(inputs, mm_mode),
                               core_ids=list(range(N_CORES)), trace=trace)
    out = np.stack([res.results[i]["out"] for i in range(N_CORES)], axis=0)
    return out, res.exec_time_ns


def kernel(**inputs) -> np.ndarray:
    out, _ = run(inputs, trace=False)
    return out


# revision 12
# speedup vs baseline: 1.0807x; 1.0807x over previous
"""Multi-head attention kernel for Trainium2, SPMD across 8 NeuronCores.

Problem: x[8,16,256,384] -> attention(8 heads, head_dim 64) -> [8,16,256,384]
Sharding: data-parallel over batch b (1 batch element per core, weights
replicated). Each core processes 16 independent slices of [256 tokens, 384],
handled in pairs ("superslices") so the QKV matmuls stream N=512.

Per-slice dataflow (activations kept feature-major, i.e. transposed):
  xT[d,t]   = DMA-transpose load of x slice         [384, 256] (bf16)
  qkT[e,t]  = w_qkv[:,e].T @ xT  (e in 0..1023)     q^T,k^T feature-major
  v[t,e]    = xT.T @ w_qkv[:, 1024:1536]            natural layout
  sT[j,i]   = k_h^T.T @ q_h^T   (per head, K=64; head pairs row-tiled)
  pT        = exp(sT / 8)                           [j, 2 heads x i]
  rowsum    = ones.T @ pT                           [1, 512] matmul
  o         = v_h.T @ pT  (head pair col-tiled into one [128,256] PSUM)
  oT        = o * broadcast(1/rowsum)               one mul per head pair
  out[t,:]  = oT.T @ w_out + b_out (bias via K=1 ones matmul)
"""

import sys
import types

sys.path.insert(0, "/opt/trn_rl_repo")

import numpy as np

import concourse.bass as bass
import concourse.bacc as bacc
import concourse.mybir as mybir
import concourse.tile as tile
from concourse.bass_utils import run_bass_kernel_spmd

N_CORES = 8
B, P, N, D = 8, 16, 256, 384
H, HD = 8, 64
INNER = H * HD  # 512
SCALE = HD ** -0.5
F32 = mybir.dt.float32

MM_MODE = "bf16"  # "bf16" | "f32r" | "f32"


def _mdt(mm_mode):
    return {"bf16": mybir.dt.bfloat16,
            "f32r": mybir.dt.float32r,
            "f32": F32}[mm_mode]


def _np_mdt(mm_mode):
    if mm_mode == "bf16":
        import ml_dtypes
        return ml_dtypes.bfloat16
    return np.float32


def _register_ntff_hook():
    """Make trace=True work under axon when antenv.axon_hooks is absent."""
    if "antenv.axon_hooks" in sys.modules:
        return
    try:
        from trn_agent_boot.trn_boot import _ntff_profile_via_ctypes
    except ImportError:
        return
    hook = _ntff_profile_via_ctypes("/opt/axon/libaxon_pjrt.so")
    mod = types.ModuleType("antenv.axon_hooks")
    mod.get_axon_ntff_profile_hook = lambda: hook
    sys.modules["antenv.axon_hooks"] = mod


def build(mm_mode=MM_MODE):
    nc = bacc.Bacc("TRN2", target_bir_lowering=False, debug=False,
                   num_devices=N_CORES)
    MDT = _mdt(mm_mode)
    x_ext = nc.declare_dram_parameter("x", [P, N, D], MDT, isOutput=False)
    wq_ext = nc.declare_dram_parameter("w_qkv", [D, 3 * INNER], MDT,
                                       isOutput=False)
    wo_ext = nc.declare_dram_parameter("w_out", [INNER, D], MDT,
                                       isOutput=False)
    bo_ext = nc.declare_dram_parameter("b_out", [D], MDT, isOutput=False)
    out_ext = nc.declare_dram_parameter("out", [P, N, D], F32, isOutput=True)

    Exp = mybir.ActivationFunctionType.Exp
    memset_dt = F32 if mm_mode != "bf16" else MDT

    with tile.TileContext(nc) as tc:
        with (
            tc.tile_pool(name="const", bufs=1) as const,
            tc.tile_pool(name="xt", bufs=2) as xt_pool,
            tc.tile_pool(name="qk", bufs=2) as qk_pool,
            tc.tile_pool(name="vp", bufs=2) as v_pool,
            tc.tile_pool(name="pt", bufs=6) as p_pool,
            tc.tile_pool(name="ot", bufs=3) as ot_pool,
            tc.tile_pool(name="ob", bufs=3) as ob_pool,
            tc.tile_pool(name="rs", bufs=6) as rs_pool,
            tc.tile_pool(name="bc", bufs=6) as bc_pool,
            tc.tile_pool(name="mmps", bufs=2, space="PSUM") as mm_ps,
            tc.tile_pool(name="sps", bufs=2, space="PSUM") as s_ps,
            tc.tile_pool(name="ops", bufs=2, space="PSUM") as o_ps,
            tc.tile_pool(name="rps", bufs=1, space="PSUM") as r_ps,
            tc.tile_pool(name="bps", bufs=1, space="PSUM") as b_ps,
        ):
            # ---- constants (loaded once) ----
            w_sb = const.tile([128, 3 * 1536], MDT, tag="w_sb")
            for kc in range(3):
                nc.sync.dma_start(w_sb[:, kc * 1536:(kc + 1) * 1536],
                                  wq_ext.ap()[kc * 128:(kc + 1) * 128, :])
            wo_sb = const.tile([128, 4 * 384], MDT, tag="wo_sb")
            for kc in range(4):
                nc.sync.dma_start(wo_sb[:, kc * 384:(kc + 1) * 384],
                                  wo_ext.ap()[kc * 128:(kc + 1) * 128, :])
            bt_sb = const.tile([1, 384], MDT, tag="bt_sb")
            nc.sync.dma_start(bt_sb[:], bo_ext.ap().unsqueeze(0))
            on_sb = const.tile([128, 128], MDT, tag="on_sb")
            nc.gpsimd.memset(on_sb[:], 1.0)
            ind0 = const.tile([1, 128], F32, tag="ind0")
            nc.gpsimd.memset(ind0[:, 0:64], 1.0)
            nc.gpsimd.memset(ind0[:, 64:128], 0.0)
            ind1 = const.tile([1, 128], F32, tag="ind1")
            nc.gpsimd.memset(ind1[:, 0:64], 0.0)
            nc.gpsimd.memset(ind1[:, 64:128], 1.0)

            # m-chunk order: interleave q and k chunks so head-pair c has
            # its q (m=c) and k (m=4+c) chunks available early.
            m_order = [0, 4, 1, 5, 2, 6, 3, 7]

            for u in range(P // 2):  # superslice of 2 token slices
                # ---- xT via DMA transpose: [256,128] dram -> [128,256] ----
                xt = xt_pool.tile([128, 3 * 512], MDT, tag="xt")
                for a in range(2):
                    for kc in range(3):
                        nc.scalar.dma_start(
                            xt[:, kc * 512 + a * 256: kc * 512 + (a + 1) * 256],
                            x_ext.ap()[2 * u + a, :, kc * 128:(kc + 1) * 128],
                            transpose=True)

                # ---- qkT chunks m (features m*128..m*128+127) ----
                qk = qk_pool.tile([128, 8 * 512], MDT, tag="qk")
                for mi, m in enumerate(m_order):
                    ps = mm_ps.tile([128, 512], F32, tag="mmps")
                    for kc in range(3):
                        nc.tensor.matmul(
                            ps[:],
                            w_sb[:, kc * 1536 + m * 128:
                                 kc * 1536 + (m + 1) * 128],
                            xt[:, kc * 512:(kc + 1) * 512],
                            start=(kc == 0), stop=(kc == 2))
                    if mi % 2 == 0:
                        nc.scalar.copy(qk[:, m * 512:(m + 1) * 512], ps[:])
                    else:
                        nc.vector.tensor_copy(qk[:, m * 512:(m + 1) * 512],
                                              ps[:])

                # ---- v natural [token, 512] per (slice, tok-chunk) ----
                v = v_pool.tile([128, 4 * 512], MDT, tag="v")
                for a in range(2):
                    for t in range(2):
                        ps = mm_ps.tile([128, 512], F32, tag="mmps")
                        for kc in range(3):
                            nc.tensor.matmul(
                                ps[:],
                                xt[:, kc * 512 + a * 256 + t * 128:
                                   kc * 512 + a * 256 + (t + 1) * 128],
                                w_sb[:, kc * 1536 + 1024: kc * 1536 + 1536],
                                start=(kc == 0), stop=(kc == 2))
                        nc.vector.tensor_copy(
                            v[:, (a * 2 + t) * 512:(a * 2 + t + 1) * 512],
                            ps[:])

                # ---- attention: head pairs (2c, 2c+1) per slice ----
                for a in range(2):
                    ot = ot_pool.tile([128, 4 * 256], MDT, tag="ot")
                    for c in range(4):
                        pts = []
                        for jc in range(2):
                            pt = p_pool.tile([128, 512], MDT, tag="pt")
                            for e in range(2):
                                sps = s_ps.tile([128, 256], F32, tag="sps")
                                nc.tensor.matmul(
                                    sps[:],
                                    qk[e * 64:e * 64 + 64,
                                       (4 + c) * 512 + a * 256 + jc * 128:
                                       (4 + c) * 512 + a * 256 + (jc + 1) * 128],
                                    qk[e * 64:e * 64 + 64,
                                       c * 512 + a * 256: c * 512 + (a + 1) * 256],
                                    start=True, stop=True,
                                    tile_position=(e * 64, 0))
                                nc.scalar.activation(
                                    pt[:, e * 256:(e + 1) * 256], sps[:], Exp,
                                    scale=SCALE)
                            pts.append(pt)
                        # rowsums for both heads: [1, 512] = ones.T @ pT
                        rps = r_ps.tile([1, 512], F32, tag="rps")
                        for jc in range(2):
                            nc.tensor.matmul(
                                rps[:], on_sb[:, 0:1], pts[jc][:],
                                start=(jc == 0), stop=(jc == 1))
                        # AV pair, col-tiled into one PSUM tile
                        ops = o_ps.tile([128, 256], F32, tag="ops")
                        for jc in range(2):
                            for e in range(2):
                                h = 2 * c + e
                                nc.tensor.matmul(
                                    ops[e * 64:(e + 1) * 64, :],
                                    v[:, (a * 2 + jc) * 512 + h * 64:
                                      (a * 2 + jc) * 512 + (h + 1) * 64],
                                    pts[jc][:, e * 256:(e + 1) * 256],
                                    start=(jc == 0), stop=(jc == 1),
                                    tile_position=(0, e * 64))
                        rs = rs_pool.tile([1, 512], F32, tag="rs")
                        nc.vector.reciprocal_approx_fast(rs[:], rps[:])
                        bps = b_ps.tile([128, 256], F32, tag="bps")
                        nc.tensor.matmul(bps[:], ind0[:], rs[0:1, 0:256],
                                         start=True, stop=False)
                        nc.tensor.matmul(bps[:], ind1[:], rs[0:1, 256:512],
                                         start=False, stop=True)
                        bc = bc_pool.tile([128, 256], F32, tag="bc")
                        nc.scalar.copy(bc[:], bps[:])
                        nc.vector.tensor_mul(ot[:, c * 256:(c + 1) * 256],
                                             ops[:], bc[:])

                    # ---- output projection; bias via K=1 ones matmul ----
                    for t in range(2):
                        fps = mm_ps.tile([128, 512], F32, tag="mmps")
                        for kc in range(4):
                            nc.tensor.matmul(
                                fps[:, 0:384],
                                ot[:, kc * 256 + t * 128:
                                   kc * 256 + (t + 1) * 128],
                                wo_sb[:, kc * 384:(kc + 1) * 384],
                                start=(kc == 0), stop=False)
                        nc.tensor.matmul(
                            fps[:, 0:384], on_sb[0:1, 0:128], bt_sb[:],
                            start=False, stop=True)
                        ob = ob_pool.tile([128, 384], F32, tag="ob")
                        nc.scalar.copy(ob[:], fps[:, 0:384])
                        nc.sync.dma_start(
                            out_ext.ap()[2 * u + a, t * 128:(t + 1) * 128, :],
                            ob[:])
    nc.compile()
    return nc


_CACHE = {}


def _get_nc(mm_mode=MM_MODE):
    if mm_mode not in _CACHE:
        _CACHE[mm_mode] = build(mm_mode)
    return _CACHE[mm_mode]


def _in_maps(inputs, mm_mode=MM_MODE):
    ndt = _np_mdt(mm_mode)
    x = np.asarray(inputs["x"]).astype(ndt)
    w_qkv = np.asarray(inputs["w_qkv"]).astype(ndt)
    w_out = np.asarray(inputs["w_out"]).astype(ndt)
    b_out = np.asarray(inputs["b_out"]).astype(ndt)
    return [
        {"x": np.ascontiguousarray(x[i]), "w_qkv": w_qkv, "w_out": w_out,
         "b_out": b_out}
        for i in range(N_CORES)
    ]


def run(inputs, trace=False, mm_mode=MM_MODE):
    """Returns (output [8,16,256,384], exec_time_ns or None)."""
    if trace:
        _register_ntff_hook()
    nc = _get_nc(mm_mode)
    res = run_bass_kernel_spmd(nc, _in_maps(inputs, mm_mode),
                               core_ids=list(range(N_CORES)), trace=trace)
    out = np.stack([res.results[i]["out"] for i in range(N_CORES)], axis=0)
    return out, res.exec_time_ns


def kernel(**inputs) -> np.ndarray:
    out, _ = run(inputs, trace=False)
    return out
